# revision 25
# baseline (speedup 1.0000x reference)
"""DiffConv (graph diffusion convolution) Trainium2 kernel, v13.

Math (reference):
    out = sum_{k=0..2} A^k @ (H @ Wf[k]) + (A^T)^k @ (H @ Wb[k]) + bias
with H [b=8, t=24, n=1024, d=64], A [t, n, n], Wf/Wb [3, d, d].

Horner per t (projections U0,U1,U2,V1,V2 = H@W* computed on HOST):
    S_f = U1 + A @ U2          S_b = V1 + A^T @ V2
    out = U0 + A @ S_f + A^T @ S_b

Changes over the v6 baseline (113.5 us -> ~102.8 us):
  * A is MEAN-CENTERED on host (Ac = A - mean_t); the rank-1 all-ones
    corrections fold into shipped U1'/V1'/U0' for free.  Halves the
    fp8 quantization error of the A operand.
  * U1/V1 shipped fp8 (x16) instead of bf16: 6 MB/t instead of 7.
    Their quantization error passes through A (row-sums ~0.5 on a
    zero-mean vector -> ~30x shrink), so the output impact is tiny.
  * Prologue in strict NEED-ORDER on the single sync-queue HWDGE
    (14 right-sized pieces; each dma_start costs ~0.6-1.2 us of queue
    time on ring credits, so piece count matters as much as bytes).
    No gpsimd/SWDGE transfers in the prologue window - they have no
    ordering vs HWDGE and steal ~40% of DMA bandwidth (v6's bug;
    first matmul fired at 17 us instead of ~10 us).
  * T_f starts with a q0-sweep across all 8 PSUM banks (8 matmuls per
    arriving j-pair piece) so the PE cannot outrun the t=0 DMA ramp;
    the q1..3 per-i passes space DVE drains ~675 ns apart.
  * 8 dummy matmuls on a memset scratch tile right after the engine
    preamble pre-warm the HAM clock gate (PE is held at 1.2 GHz until
    ~3.4 us of sustained activity) during the dead DMA-init window.
  * PSUM pool uses all 8 banks.

All spmm matmuls in fp8e4 with perf_mode=DoubleRow (contracts 2
K-planes per instruction via 3D APs [128, 2, free]; 216 ns warm
back-to-back spacing per 256-deep 512-wide matmul = the practical fp8
peak, 384 matmuls/core = 82.9 us PE floor).  The PE runs back-to-back
so the HAM clock gate stays at 2.4 GHz.

Drains are scalar_tensor_tensor on DVE: S8 = psum*2^-17 + U1'x16 (fp8),
osb = psum*2^-21 + U0' (bf16).  Scales (exact powers of two):
Ac8 = Ac*2^17 (|Ac|<2^-11 so |Ac8|<64), U1/U2/V1/V2 shipped x16,
U0' shipped unscaled bf16.

Sharding: t across 8 cores (3 each), zero collectives.
"""

import os
import sys

sys.path.insert(0, "/opt/trn_rl_repo")

import ml_dtypes
import numpy as np

import concourse.tile as tile
from concourse import bacc, mybir
from concourse.bass_utils import run_bass_kernel_spmd

B, T, N, D = 8, 24, 1024, 64
NCORES = 8
TPC = T // NCORES  # t-steps per core
NB = N // 128  # 128-row blocks of n
F32 = mybir.dt.float32
BF16 = mybir.dt.bfloat16
FP8 = mybir.dt.float8e4
BD = B * D
DR = mybir.MatmulPerfMode.DoubleRow
MULT = mybir.AluOpType.mult
ADD = mybir.AluOpType.add

SC_A = float(2.0**17)  # Ac8 = Ac * SC_A
SC_U = 16.0  # U1/V1/U2/V2 shipped * SC_U
C_S = float(2.0**-17)  # S8 = psum * C_S + U1x16  (= 16*S)
C_O = float(2.0**-21)  # osb = psum * C_O + U0

_cached = {}


def _build():
    if "nc" in _cached:
        return _cached["nc"]

    nc = bacc.Bacc("TRN2", target_bir_lowering=False, debug=False)
    # Host-pre-permuted layouts (see prep_in_maps).
    dAF = nc.dram_tensor("AFP", [TPC, 128, 2, NB, N], FP8, kind="ExternalInput")
    dUV8 = nc.dram_tensor("UV8P", [TPC, 128, 2, NB, BD], FP8, kind="ExternalInput")
    dUV1 = nc.dram_tensor("UV1P", [TPC, 128, 2, NB, BD], FP8, kind="ExternalInput")
    dU0 = nc.dram_tensor("U0P", [TPC, 128, NB, BD], BF16, kind="ExternalInput")
    dOUT = nc.dram_tensor("out", [TPC, 128, NB, BD], BF16, kind="ExternalOutput")

    with tile.TileContext(nc) as tc:
        with (
            tc.tile_pool(name="amat", bufs=2) as apool,
            tc.tile_pool(name="uv8", bufs=2) as uv8pool,
            tc.tile_pool(name="uv1", bufs=2) as uv1pool,
            tc.tile_pool(name="u0t", bufs=2) as u0pool,
            tc.tile_pool(name="sfb", bufs=2) as spool,
            tc.tile_pool(name="osb", bufs=2) as opool,
            tc.tile_pool(name="sps", bufs=8, space="PSUM") as sps,
        ):
            afs, uv8s, uv1s, u0s = {}, {}, {}, {}

            # ---------------- PE pre-warm --------------------------------
            # The HAM clock gate holds the PE at 1.2 GHz until it has been
            # busy ~3.4 us; real data lands ~4.5 us after the engine
            # preamble.  Burn the dead window on 8 dummy matmuls against a
            # memset scratch tile (ending right as the first real piece
            # arrives) so the real stream runs at 2.4 GHz from the start.
            warm = apool.tile([128, 2, BD], FP8, tag="warm", name="warmup")
            gate = apool.tile([128, 4], FP8, tag="gate", name="gate")
            wps = sps.tile([128, BD], F32, tag="sps", name="warmps")
            nc.gpsimd.memset(warm[:], 0.0)
            for _ in range(8):
                nc.tensor.matmul(
                    wps[:],
                    warm[:, :, 0:128],
                    warm[:],
                    start=True,
                    stop=True,
                    perf_mode=DR,
                )

            def alloc_t(t):
                afs[t] = apool.tile([128, 2, NB, N], FP8, tag="af", name=f"af{t}")
                uv8s[t] = uv8pool.tile(
                    [128, 2, NB, BD], FP8, tag="uv8", name=f"uv8{t}"
                )
                uv1s[t] = uv1pool.tile(
                    [128, 2, NB, BD], FP8, tag="uv1", name=f"uv1{t}"
                )
                u0s[t] = u0pool.tile([128, NB, BD], BF16, tag="u0", name=f"u0{t}")

            # ---------------- prologue: t=0 strictly in need-order --------
            # ONE in-order HWDGE stream (sync queue) so no later tensor can
            # steal DMA bandwidth from an earlier-needed piece.  The first
            # T_f matmul group fires after just uv8-piece0 + a 32 KB af head
            # (the i=0 columns of j-pair 0); T_f reuses the same 1.5 MB for
            # all 8 i-groups, so DMA races ahead into the backward half
            # during T_f.
            alloc_t(0)
            # head: exactly what MM(dir0, i=0, q=0) reads -> earliest start
            # Each sync-queue dma_start occupies the queue ~0.6-0.75 us
            # (issue + HWDGE ring credits), so the piece COUNT is as
            # costly as the bytes: 14 pieces total, sized so each lands
            # just before its first consumer.
            nc.sync.dma_start(
                uv8s[0][:, 0, 0:2], dUV8.ap()[0, :, 0, 0:2]
            )
            nc.sync.dma_start(
                afs[0][:, 0, 0:2, 0:128], dAF.ap()[0, :, 0, 0:2, 0:128]
            )
            # rest of j-pair 0 — the q0-sweep walks i=0..7 through it
            nc.sync.dma_start(
                afs[0][:, 0, 0:2, 128:], dAF.ap()[0, :, 0, 0:2, 128:]
            )
            nc.sync.dma_start(uv8s[0][:, 0, 2:], dUV8.ap()[0, :, 0, 2:])
            for q in range(1, NB // 2):
                nc.sync.dma_start(
                    afs[0][:, 0, 2 * q : 2 * q + 2],
                    dAF.ap()[0, :, 0, 2 * q : 2 * q + 2],
                )
            # The rest of t=0 (uv1f + backward half + u0) rides the SCALAR
            # engine's independent HWDGE queue: each queue's dma_starts
            # serialize at ~1.2 us apiece on ring credits, so one queue
            # cannot drain 6 MB before T_b needs its data.  A tiny Act
            # copy reading an af0rest cell gates this queue on the
            # critical forward pieces having landed (so it cannot steal
            # bandwidth from them); explicit dep edges stop the Tile
            # scheduler from hoisting the dma_starts above the copy.
            gatec = nc.scalar.copy(gate[:], afs[0][:, 0, 1, 128:132])
            sdmas = [
                nc.scalar.dma_start(uv1s[0][:, 0], dUV1.ap()[0, :, 0]),
                nc.scalar.dma_start(uv8s[0][:, 1, 0:4], dUV8.ap()[0, :, 1, 0:4]),
                nc.scalar.dma_start(afs[0][:, 1, 0:4], dAF.ap()[0, :, 1, 0:4]),
                nc.scalar.dma_start(uv8s[0][:, 1, 4:], dUV8.ap()[0, :, 1, 4:]),
                nc.scalar.dma_start(afs[0][:, 1, 4:], dAF.ap()[0, :, 1, 4:]),
                nc.scalar.dma_start(uv1s[0][:, 1], dUV1.ap()[0, :, 1]),
                nc.scalar.dma_start(u0s[0][:], dU0.ap()[0]),
            ]
            _gi = gatec.ins if hasattr(gatec, "ins") else gatec
            for _d in sdmas:
                tile.add_dep_helper(
                    _gi,
                    _d.ins if hasattr(_d, "ins") else _d,
                    sync=False,
                    reason="t0 scalar-queue DMAs must not start before the "
                    "critical forward pieces have landed",
                )

            for t in range(TPC):
                af, uv8, uv1, u0 = afs[t], uv8s[t], uv1s[t], u0s[t]
                osb = opool.tile([128, NB, BD], BF16, tag="osb")
                sfb = spool.tile([128, 2, NB, BD], FP8, tag="sfb")
                have_next = t + 1 < TPC
                if have_next:
                    alloc_t(t + 1)

                # ---- T_f: S8[0] = 16*(U1' + A @ U2) -------------------
                # q0 is a sweep over all 8 PSUM banks: 8 matmuls per
                # arriving j-pair piece, so even a warm (2.4 GHz) PE can't
                # outrun the t=0 DMA ramp; the q1..3 per-i passes then
                # space the drains ~675 ns apart (>= DVE drain time), so
                # the DVE never backlogs into T_b/FB.
                psf = [
                    sps.tile([128, BD], F32, tag="sps", name=f"psf{t}_{i}")
                    for i in range(NB)
                ]
                for i in range(NB):
                    nc.tensor.matmul(
                        psf[i][:],
                        af[:, 0, 0:2, i * 128 : (i + 1) * 128],
                        uv8[:, 0, 0:2, :],
                        start=True,
                        stop=False,
                        perf_mode=DR,
                    )
                for i in range(NB):
                    for q in range(1, NB // 2):
                        nc.tensor.matmul(
                            psf[i][:],
                            af[:, 0, 2 * q : 2 * q + 2, i * 128 : (i + 1) * 128],
                            uv8[:, 0, 2 * q : 2 * q + 2, :],
                            start=False,
                            stop=(q == NB // 2 - 1),
                            perf_mode=DR,
                        )
                    nc.vector.scalar_tensor_tensor(
                        sfb[:, 0, i], psf[i][:], C_S, uv1[:, 0, i], MULT, ADD
                    )

                # ---- T_b: S8[1] = 16*(V1' + A^T @ V2) -----------------
                for i in range(NB):
                    if i == 0 and have_next:
                        # t+1 prefetch rides the SAME in-order HWDGE
                        # queue: it cannot start before t's (and t=0
                        # prologue's) earlier-needed pieces finish.
                        # (gpsimd/SWDGE has no ordering vs HWDGE and
                        # was measured stealing ~40% of prologue BW.)
                        nc.sync.dma_start(afs[t + 1][:], dAF.ap()[t + 1])
                        nc.sync.dma_start(
                            uv8s[t + 1][:, 0], dUV8.ap()[t + 1, :, 0]
                        )
                        nc.sync.dma_start(
                            uv1s[t + 1][:, 0], dUV1.ap()[t + 1, :, 0]
                        )
                    ps = sps.tile([128, BD], F32, tag="sps")
                    for q in range(NB // 2):
                        nc.tensor.matmul(
                            ps[:],
                            af[:, 1, 2 * q : 2 * q + 2, i * 128 : (i + 1) * 128],
                            uv8[:, 1, 2 * q : 2 * q + 2, :],
                            start=(q == 0),
                            stop=(q == NB // 2 - 1),
                            perf_mode=DR,
                        )
                    nc.vector.scalar_tensor_tensor(
                        sfb[:, 1, i], ps[:], C_S, uv1[:, 1, i], MULT, ADD
                    )

                # ---- FB: osb = U0' + A @ S_f + A^T @ S_b ----
                for i in range(NB):
                    if i == 0 and have_next:
                        nc.sync.dma_start(uv8s[t + 1][:, 1], dUV8.ap()[t + 1, :, 1])
                        nc.sync.dma_start(uv1s[t + 1][:, 1], dUV1.ap()[t + 1, :, 1])
                        nc.sync.dma_start(u0s[t + 1][:], dU0.ap()[t + 1])
                    ps = sps.tile([128, BD], F32, tag="sps")
                    for j in range(NB):
                        nc.tensor.matmul(
                            ps[:],
                            af[:, :, j, i * 128 : (i + 1) * 128],
                            sfb[:, :, j, :],
                            start=(j == 0),
                            stop=(j == NB - 1),
                            perf_mode=DR,
                        )
                    nc.vector.scalar_tensor_tensor(
                        osb[:, i], ps[:], C_O, u0[:, i], MULT, ADD
                    )
                    # store incrementally so the kernel tail only waits on
                    # the last 1-2 blocks
                    if i == 3:
                        nc.sync.dma_start(dOUT.ap()[t, :, 0:4], osb[:, 0:4])
                    elif i == 5:
                        nc.sync.dma_start(dOUT.ap()[t, :, 4:6], osb[:, 4:6])
                    elif i == 6:
                        nc.sync.dma_start(dOUT.ap()[t, :, 6:7], osb[:, 6:7])
                    elif i == 7:
                        nc.sync.dma_start(dOUT.ap()[t, :, 7:8], osb[:, 7:8])

    nc.compile()
    _cached["nc"] = nc
    return nc


def _uvperm(X):
    """[b, t(core-slice), n, d] -> [t, 128, NB, B*D] with
    out[t, p, i, b*64+d] = X[b, t, i*128+p, d]."""
    tpc = X.shape[1]
    return np.ascontiguousarray(
        X.transpose(1, 2, 0, 3)
        .reshape(tpc, NB, 128, B, D)
        .transpose(0, 2, 1, 3, 4)
        .reshape(tpc, 128, NB, BD)
    )


def _prep_core(UVall, A8, AT8, U0, c):
    ts = slice(c * TPC, (c + 1) * TPC)
    # AFP[t, p, dir, j, c] = (dir==0 ? Ac^T : Ac)[j*128+p, c] * 2^17 (fp8)
    AF = np.stack(
        [
            AT8[ts].reshape(TPC, NB, 128, N),
            A8[ts].reshape(TPC, NB, 128, N),
        ],
        axis=2,
    )  # [t, j, dir, p, col]
    AF = np.ascontiguousarray(AF.transpose(0, 3, 2, 1, 4))  # [t, p, dir, j, col]
    U1, U2, V1, V2 = (UVall[k][:, ts] for k in range(4))
    f8 = mybir.dt.np(FP8)
    # stack at axis=2: [t, 128, 2(slot), NB, BD]
    UV8 = np.ascontiguousarray(np.stack([_uvperm(U2), _uvperm(V2)], axis=2))
    UV1 = np.ascontiguousarray(np.stack([_uvperm(U1), _uvperm(V1)], axis=2))
    U0P = _uvperm(U0[:, ts])
    bf = ml_dtypes.bfloat16
    return {
        "AFP": AF,
        "UV8P": UV8.astype(f8),
        "UV1P": UV1.astype(f8),
        "U0P": U0P.astype(bf),
    }


def prep_in_maps(H, A, Wf, Wb, bias):
    H = np.ascontiguousarray(np.asarray(H, dtype=np.float32))
    A = np.ascontiguousarray(np.asarray(A, dtype=np.float32))
    Wf = np.asarray(Wf, dtype=np.float32)
    Wb = np.asarray(Wb, dtype=np.float32)
    bias = np.asarray(bias, dtype=np.float32)

    # ---- mean-center A; the all-ones rank-1 part folds into U1/V1/U0 ----
    mA = A.mean(axis=(1, 2), keepdims=True)  # [T,1,1]
    Ac = A - mA  # zero-mean, |Ac| < 1/N
    mAt = mA[:, 0, 0]  # [T]

    f8 = mybir.dt.np(FP8)
    A8 = (Ac * SC_A).astype(f8)
    AT8 = np.ascontiguousarray((Ac * SC_A).transpose(0, 2, 1)).astype(f8)

    U0 = (H @ (Wf[0] + Wb[0]) + bias).astype(np.float32)  # unscaled
    U1 = H @ Wf[1]
    U2 = H @ Wf[2]
    V1 = H @ Wb[1]
    V2 = H @ Wb[2]

    # Rank-1 corrections (J = all-ones):  A = mA*J + Ac
    #   S_f = U1 + A U2  = (U1 + mA*colsum(U2)) + Ac U2     -> ship U1'
    #   S_b = V1 + A^T V2 = (V1 + mA*colsum(V2)) + Ac^T V2  -> ship V1'
    #   out += mA*(colsum(S_f) + colsum(S_b))               -> fold into U0'
    # colsum(S_f) = colsum(U1') + (1^T Ac) @ U2  (host, exact f32)
    csU2 = U2.sum(axis=2)  # [B,T,D]
    csV2 = V2.sum(axis=2)
    mAb = mAt[None, :, None]  # [1,T,1]
    U1p = U1 + (mAb * csU2)[:, :, None, :]
    V1p = V1 + (mAb * csV2)[:, :, None, :]
    colAc = Ac.sum(axis=1)  # [T,N]: (1^T Ac)_k = sum_j Ac[j,k]
    rowAc = Ac.sum(axis=2)  # [T,N]: (Ac 1)_i  = sum_j Ac[i,j]
    csSf = U1p.sum(axis=2) + np.einsum("tk,btkd->btd", colAc, U2)
    csSb = V1p.sum(axis=2) + np.einsum("ti,btid->btd", rowAc, V2)
    U0p = U0 + (mAb * (csSf + csSb))[:, :, None, :]

    UVall = [
        (X * SC_U).astype(np.float32) for X in (U1p, U2, V1p, V2)
    ]  # U1', U2, V1', V2 (x16)

    return [_prep_core(UVall, A8, AT8, U0p, c) for c in range(NCORES)]


def _postprocess(res):
    # osb = psum*2^-21 + U0' is already the final unscaled output.
    outp = np.concatenate(
        [np.asarray(res.results[c]["out"]) for c in range(NCORES)], axis=0
    ).astype(np.float32)
    out = (
        outp.reshape(T, 128, NB, B, D)
        .transpose(3, 0, 2, 1, 4)  # [b, t, i, p, d]
        .reshape(B, T, N, D)
    )
    return np.ascontiguousarray(out)


def kernel(H, A, Wf, Wb, bias):
    nc = _build()
    in_maps = prep_in_maps(H, A, Wf, Wb, bias)
    res = run_bass_kernel_spmd(nc, in_maps, core_ids=list(range(NCORES)))
    return _postprocess(res)


# revision 26
# speedup vs baseline: 1.2420x; 1.2420x over previous
"""DiffConv (graph diffusion convolution) Trainium2 kernel, v13.

Math (reference):
    out = sum_{k=0..2} A^k @ (H @ Wf[k]) + (A^T)^k @ (H @ Wb[k]) + bias
with H [b=8, t=24, n=1024, d=64], A [t, n, n], Wf/Wb [3, d, d].

Horner per t (projections U0,U1,U2,V1,V2 = H@W* computed on HOST):
    S_f = U1 + A @ U2          S_b = V1 + A^T @ V2
    out = U0 + A @ S_f + A^T @ S_b

Changes over the v6 baseline (113.5 us -> ~102.8 us):
  * A is MEAN-CENTERED on host (Ac = A - mean_t); the rank-1 all-ones
    corrections fold into shipped U1'/V1'/U0' for free.  Halves the
    fp8 quantization error of the A operand.
  * U1/V1 shipped fp8 (x16) instead of bf16: 6 MB/t instead of 7.
    Their quantization error passes through A (row-sums ~0.5 on a
    zero-mean vector -> ~30x shrink), so the output impact is tiny.
  * Prologue in strict NEED-ORDER on the single sync-queue HWDGE
    (14 right-sized pieces; each dma_start costs ~0.6-1.2 us of queue
    time on ring credits, so piece count matters as much as bytes).
    No gpsimd/SWDGE transfers in the prologue window - they have no
    ordering vs HWDGE and steal ~40% of DMA bandwidth (v6's bug;
    first matmul fired at 17 us instead of ~10 us).
  * T_f starts with a q0-sweep across all 8 PSUM banks (8 matmuls per
    arriving j-pair piece) so the PE cannot outrun the t=0 DMA ramp;
    the q1..3 per-i passes space DVE drains ~675 ns apart.
  * 8 dummy matmuls on a memset scratch tile right after the engine
    preamble pre-warm the HAM clock gate (PE is held at 1.2 GHz until
    ~3.4 us of sustained activity) during the dead DMA-init window.
  * PSUM pool uses all 8 banks.

All spmm matmuls in fp8e4 with perf_mode=DoubleRow (contracts 2
K-planes per instruction via 3D APs [128, 2, free]; 216 ns warm
back-to-back spacing per 256-deep 512-wide matmul = the practical fp8
peak, 384 matmuls/core = 82.9 us PE floor).  The PE runs back-to-back
so the HAM clock gate stays at 2.4 GHz.

Drains are scalar_tensor_tensor on DVE: S8 = psum*2^-17 + U1'x16 (fp8),
osb = psum*2^-21 + U0' (bf16).  Scales (exact powers of two):
Ac8 = Ac*2^17 (|Ac|<2^-11 so |Ac8|<64), U1/U2/V1/V2 shipped x16,
U0' shipped unscaled bf16.

Sharding: t across 8 cores (3 each), zero collectives.
"""

import os
import sys

sys.path.insert(0, "/opt/trn_rl_repo")

import ml_dtypes
import numpy as np

import concourse.tile as tile
from concourse import bacc, mybir
from concourse.bass_utils import run_bass_kernel_spmd

B, T, N, D = 8, 24, 1024, 64
NCORES = 8
TPC = T // NCORES  # t-steps per core
NB = N // 128  # 128-row blocks of n
F32 = mybir.dt.float32
BF16 = mybir.dt.bfloat16
FP8 = mybir.dt.float8e4
BD = B * D
DR = mybir.MatmulPerfMode.DoubleRow
MULT = mybir.AluOpType.mult
ADD = mybir.AluOpType.add

SC_A = float(2.0**17)  # Ac8 = Ac * SC_A
SC_U = 16.0  # U1/V1/U2/V2 shipped * SC_U
C_S = float(2.0**-17)  # S8 = psum * C_S + U1x16  (= 16*S)
C_O = float(2.0**-21)  # osb = psum * C_O + U0

_cached = {}


def _build():
    if "nc" in _cached:
        return _cached["nc"]

    nc = bacc.Bacc("TRN2", target_bir_lowering=False, debug=False)
    # Host-pre-permuted layouts (see prep_in_maps).
    dAF = nc.dram_tensor("AFP", [TPC, 128, 2, NB, N], FP8, kind="ExternalInput")
    dUV8 = nc.dram_tensor("UV8P", [TPC, 128, 2, NB, BD], FP8, kind="ExternalInput")
    dUV1 = nc.dram_tensor("UV1P", [TPC, 128, 2, NB, BD], FP8, kind="ExternalInput")
    dU0 = nc.dram_tensor("U0P", [TPC, 128, NB, BD], BF16, kind="ExternalInput")
    dOUT = nc.dram_tensor("out", [TPC, 128, NB, BD], BF16, kind="ExternalOutput")

    with tile.TileContext(nc) as tc:
        with (
            tc.tile_pool(name="amat", bufs=2) as apool,
            tc.tile_pool(name="uv8", bufs=2) as uv8pool,
            tc.tile_pool(name="uv1", bufs=2) as uv1pool,
            tc.tile_pool(name="u0t", bufs=2) as u0pool,
            tc.tile_pool(name="sfb", bufs=2) as spool,
            tc.tile_pool(name="osb", bufs=2) as opool,
            tc.tile_pool(name="sps", bufs=8, space="PSUM") as sps,
        ):
            afs, uv8s, uv1s, u0s = {}, {}, {}, {}

            # ---------------- PE pre-warm --------------------------------
            # The HAM clock gate holds the PE at 1.2 GHz until it has been
            # busy ~3.4 us; real data lands ~4.5 us after the engine
            # preamble.  Burn the dead window on 8 dummy matmuls against a
            # memset scratch tile (ending right as the first real piece
            # arrives) so the real stream runs at 2.4 GHz from the start.
            warm = apool.tile([128, 2, BD], FP8, tag="warm", name="warmup")
            wps = sps.tile([128, BD], F32, tag="sps", name="warmps")
            nc.gpsimd.memset(warm[:], 0.0)
            for _ in range(8):
                nc.tensor.matmul(
                    wps[:],
                    warm[:, :, 0:128],
                    warm[:],
                    start=True,
                    stop=True,
                    perf_mode=DR,
                )

            def alloc_t(t):
                afs[t] = apool.tile([128, 2, NB, N], FP8, tag="af", name=f"af{t}")
                uv8s[t] = uv8pool.tile(
                    [128, 2, NB, BD], FP8, tag="uv8", name=f"uv8{t}"
                )
                uv1s[t] = uv1pool.tile(
                    [128, 2, NB, BD], FP8, tag="uv1", name=f"uv1{t}"
                )
                u0s[t] = u0pool.tile([128, NB, BD], BF16, tag="u0", name=f"u0{t}")

            # ---------------- prologue: t=0 strictly in need-order --------
            # ONE in-order HWDGE stream (sync queue) so no later tensor can
            # steal DMA bandwidth from an earlier-needed piece.  The first
            # T_f matmul group fires after just uv8-piece0 + a 32 KB af head
            # (the i=0 columns of j-pair 0); T_f reuses the same 1.5 MB for
            # all 8 i-groups, so DMA races ahead into the backward half
            # during T_f.
            alloc_t(0)
            # head: exactly what MM(dir0, i=0, q=0) reads -> earliest start
            # Each sync-queue dma_start occupies the queue ~0.6-0.75 us
            # (issue + HWDGE ring credits), so the piece COUNT is as
            # costly as the bytes: 14 pieces total, sized so each lands
            # just before its first consumer.
            nc.sync.dma_start(
                uv8s[0][:, 0, 0:2], dUV8.ap()[0, :, 0, 0:2]
            )
            nc.sync.dma_start(
                afs[0][:, 0, 0:2, 0:128], dAF.ap()[0, :, 0, 0:2, 0:128]
            )
            # rest of j-pair 0 — the q0-sweep walks i=0..7 through it
            nc.sync.dma_start(
                afs[0][:, 0, 0:2, 128:], dAF.ap()[0, :, 0, 0:2, 128:]
            )
            nc.sync.dma_start(uv8s[0][:, 0, 2:], dUV8.ap()[0, :, 0, 2:])
            for q in range(1, NB // 2):
                nc.sync.dma_start(
                    afs[0][:, 0, 2 * q : 2 * q + 2],
                    dAF.ap()[0, :, 0, 2 * q : 2 * q + 2],
                )
            nc.sync.dma_start(uv1s[0][:, 0], dUV1.ap()[0, :, 0])  # U1'x16
            # backward half, still in need-order on the same queue
            nc.sync.dma_start(uv8s[0][:, 1, 0:4], dUV8.ap()[0, :, 1, 0:4])
            nc.sync.dma_start(
                afs[0][:, 1, 0:4], dAF.ap()[0, :, 1, 0:4]
            )
            nc.sync.dma_start(uv8s[0][:, 1, 4:], dUV8.ap()[0, :, 1, 4:])
            nc.sync.dma_start(
                afs[0][:, 1, 4:], dAF.ap()[0, :, 1, 4:]
            )
            nc.sync.dma_start(uv1s[0][:, 1], dUV1.ap()[0, :, 1])  # V1'x16
            nc.sync.dma_start(u0s[0][:], dU0.ap()[0])

            for t in range(TPC):
                af, uv8, uv1, u0 = afs[t], uv8s[t], uv1s[t], u0s[t]
                osb = opool.tile([128, NB, BD], BF16, tag="osb")
                sfb = spool.tile([128, 2, NB, BD], FP8, tag="sfb")
                have_next = t + 1 < TPC
                if have_next:
                    alloc_t(t + 1)

                # ---- T_f: S8[0] = 16*(U1' + A @ U2) -------------------
                # q0 is a sweep over all 8 PSUM banks: 8 matmuls per
                # arriving j-pair piece, so even a warm (2.4 GHz) PE can't
                # outrun the t=0 DMA ramp; the q1..3 per-i passes then
                # space the drains ~675 ns apart (>= DVE drain time), so
                # the DVE never backlogs into T_b/FB.
                psf = [
                    sps.tile([128, BD], F32, tag="sps", name=f"psf{t}_{i}")
                    for i in range(NB)
                ]
                for i in range(NB):
                    nc.tensor.matmul(
                        psf[i][:],
                        af[:, 0, 0:2, i * 128 : (i + 1) * 128],
                        uv8[:, 0, 0:2, :],
                        start=True,
                        stop=False,
                        perf_mode=DR,
                    )
                for i in range(NB):
                    for q in range(1, NB // 2):
                        nc.tensor.matmul(
                            psf[i][:],
                            af[:, 0, 2 * q : 2 * q + 2, i * 128 : (i + 1) * 128],
                            uv8[:, 0, 2 * q : 2 * q + 2, :],
                            start=False,
                            stop=(q == NB // 2 - 1),
                            perf_mode=DR,
                        )
                    nc.vector.scalar_tensor_tensor(
                        sfb[:, 0, i], psf[i][:], C_S, uv1[:, 0, i], MULT, ADD
                    )

                # ---- T_b: S8[1] = 16*(V1' + A^T @ V2) -----------------
                for i in range(NB):
                    if i == 0 and have_next:
                        # t+1 prefetch rides the SAME in-order HWDGE
                        # queue: it cannot start before t's (and t=0
                        # prologue's) earlier-needed pieces finish.
                        # (gpsimd/SWDGE has no ordering vs HWDGE and
                        # was measured stealing ~40% of prologue BW.)
                        nc.sync.dma_start(afs[t + 1][:], dAF.ap()[t + 1])
                        nc.sync.dma_start(
                            uv8s[t + 1][:, 0], dUV8.ap()[t + 1, :, 0]
                        )
                        nc.sync.dma_start(
                            uv1s[t + 1][:, 0], dUV1.ap()[t + 1, :, 0]
                        )
                    ps = sps.tile([128, BD], F32, tag="sps")
                    for q in range(NB // 2):
                        nc.tensor.matmul(
                            ps[:],
                            af[:, 1, 2 * q : 2 * q + 2, i * 128 : (i + 1) * 128],
                            uv8[:, 1, 2 * q : 2 * q + 2, :],
                            start=(q == 0),
                            stop=(q == NB // 2 - 1),
                            perf_mode=DR,
                        )
                    nc.vector.scalar_tensor_tensor(
                        sfb[:, 1, i], ps[:], C_S, uv1[:, 1, i], MULT, ADD
                    )

                # ---- FB: osb = U0' + A @ S_f + A^T @ S_b ----
                for i in range(NB):
                    if i == 0 and have_next:
                        nc.sync.dma_start(uv8s[t + 1][:, 1], dUV8.ap()[t + 1, :, 1])
                        nc.sync.dma_start(uv1s[t + 1][:, 1], dUV1.ap()[t + 1, :, 1])
                        nc.sync.dma_start(u0s[t + 1][:], dU0.ap()[t + 1])
                    ps = sps.tile([128, BD], F32, tag="sps")
                    for j in range(NB):
                        nc.tensor.matmul(
                            ps[:],
                            af[:, :, j, i * 128 : (i + 1) * 128],
                            sfb[:, :, j, :],
                            start=(j == 0),
                            stop=(j == NB - 1),
                            perf_mode=DR,
                        )
                    nc.vector.scalar_tensor_tensor(
                        osb[:, i], ps[:], C_O, u0[:, i], MULT, ADD
                    )
                    # store incrementally so the kernel tail only waits on
                    # the last 1-2 blocks
                    if i == 3:
                        nc.sync.dma_start(dOUT.ap()[t, :, 0:4], osb[:, 0:4])
                    elif i == 5:
                        nc.sync.dma_start(dOUT.ap()[t, :, 4:6], osb[:, 4:6])
                    elif i == 6:
                        nc.sync.dma_start(dOUT.ap()[t, :, 6:7], osb[:, 6:7])
                    elif i == 7:
                        nc.sync.dma_start(dOUT.ap()[t, :, 7:8], osb[:, 7:8])

    nc.compile()
    _cached["nc"] = nc
    return nc


def _uvperm(X):
    """[b, t(core-slice), n, d] -> [t, 128, NB, B*D] with
    out[t, p, i, b*64+d] = X[b, t, i*128+p, d]."""
    tpc = X.shape[1]
    return np.ascontiguousarray(
        X.transpose(1, 2, 0, 3)
        .reshape(tpc, NB, 128, B, D)
        .transpose(0, 2, 1, 3, 4)
        .reshape(tpc, 128, NB, BD)
    )


def _prep_core(UVall, A8, AT8, U0, c):
    ts = slice(c * TPC, (c + 1) * TPC)
    # AFP[t, p, dir, j, c] = (dir==0 ? Ac^T : Ac)[j*128+p, c] * 2^17 (fp8)
    AF = np.stack(
        [
            AT8[ts].reshape(TPC, NB, 128, N),
            A8[ts].reshape(TPC, NB, 128, N),
        ],
        axis=2,
    )  # [t, j, dir, p, col]
    AF = np.ascontiguousarray(AF.transpose(0, 3, 2, 1, 4))  # [t, p, dir, j, col]
    U1, U2, V1, V2 = (UVall[k][:, ts] for k in range(4))
    f8 = mybir.dt.np(FP8)
    # stack at axis=2: [t, 128, 2(slot), NB, BD]
    UV8 = np.ascontiguousarray(np.stack([_uvperm(U2), _uvperm(V2)], axis=2))
    UV1 = np.ascontiguousarray(np.stack([_uvperm(U1), _uvperm(V1)], axis=2))
    U0P = _uvperm(U0[:, ts])
    bf = ml_dtypes.bfloat16
    return {
        "AFP": AF,
        "UV8P": UV8.astype(f8),
        "UV1P": UV1.astype(f8),
        "U0P": U0P.astype(bf),
    }


def prep_in_maps(H, A, Wf, Wb, bias):
    H = np.ascontiguousarray(np.asarray(H, dtype=np.float32))
    A = np.ascontiguousarray(np.asarray(A, dtype=np.float32))
    Wf = np.asarray(Wf, dtype=np.float32)
    Wb = np.asarray(Wb, dtype=np.float32)
    bias = np.asarray(bias, dtype=np.float32)

    # ---- mean-center A; the all-ones rank-1 part folds into U1/V1/U0 ----
    mA = A.mean(axis=(1, 2), keepdims=True)  # [T,1,1]
    Ac = A - mA  # zero-mean, |Ac| < 1/N
    mAt = mA[:, 0, 0]  # [T]

    f8 = mybir.dt.np(FP8)
    A8 = (Ac * SC_A).astype(f8)
    AT8 = np.ascontiguousarray((Ac * SC_A).transpose(0, 2, 1)).astype(f8)

    U0 = (H @ (Wf[0] + Wb[0]) + bias).astype(np.float32)  # unscaled
    U1 = H @ Wf[1]
    U2 = H @ Wf[2]
    V1 = H @ Wb[1]
    V2 = H @ Wb[2]

    # Rank-1 corrections (J = all-ones):  A = mA*J + Ac
    #   S_f = U1 + A U2  = (U1 + mA*colsum(U2)) + Ac U2     -> ship U1'
    #   S_b = V1 + A^T V2 = (V1 + mA*colsum(V2)) + Ac^T V2  -> ship V1'
    #   out += mA*(colsum(S_f) + colsum(S_b))               -> fold into U0'
    # colsum(S_f) = colsum(U1') + (1^T Ac) @ U2  (host, exact f32)
    csU2 = U2.sum(axis=2)  # [B,T,D]
    csV2 = V2.sum(axis=2)
    mAb = mAt[None, :, None]  # [1,T,1]
    U1p = U1 + (mAb * csU2)[:, :, None, :]
    V1p = V1 + (mAb * csV2)[:, :, None, :]
    colAc = Ac.sum(axis=1)  # [T,N]: (1^T Ac)_k = sum_j Ac[j,k]
    rowAc = Ac.sum(axis=2)  # [T,N]: (Ac 1)_i  = sum_j Ac[i,j]
    csSf = U1p.sum(axis=2) + np.einsum("tk,btkd->btd", colAc, U2)
    csSb = V1p.sum(axis=2) + np.einsum("ti,btid->btd", rowAc, V2)
    U0p = U0 + (mAb * (csSf + csSb))[:, :, None, :]

    UVall = [
        (X * SC_U).astype(np.float32) for X in (U1p, U2, V1p, V2)
    ]  # U1', U2, V1', V2 (x16)

    return [_prep_core(UVall, A8, AT8, U0p, c) for c in range(NCORES)]


def _postprocess(res):
    # osb = psum*2^-21 + U0' is already the final unscaled output.
    outp = np.concatenate(
        [np.asarray(res.results[c]["out"]) for c in range(NCORES)], axis=0
    ).astype(np.float32)
    out = (
        outp.reshape(T, 128, NB, B, D)
        .transpose(3, 0, 2, 1, 4)  # [b, t, i, p, d]
        .reshape(B, T, N, D)
    )
    return np.ascontiguousarray(out)


def kernel(H, A, Wf, Wb, bias):
    nc = _build()
    in_maps = prep_in_maps(H, A, Wf, Wb, bias)
    res = run_bass_kernel_spmd(nc, in_maps, core_ids=list(range(NCORES)))
    return _postprocess(res)


# revision 27
# speedup vs baseline: 1.2495x; 1.0060x over previous
"""DiffConv (graph diffusion convolution) Trainium2 kernel, v13.

Math (reference):
    out = sum_{k=0..2} A^k @ (H @ Wf[k]) + (A^T)^k @ (H @ Wb[k]) + bias
with H [b=8, t=24, n=1024, d=64], A [t, n, n], Wf/Wb [3, d, d].

Horner per t (projections U0,U1,U2,V1,V2 = H@W* computed on HOST):
    S_f = U1 + A @ U2          S_b = V1 + A^T @ V2
    out = U0 + A @ S_f + A^T @ S_b

Changes over the v6 baseline (113.5 us -> ~102.8 us):
  * A is MEAN-CENTERED on host (Ac = A - mean_t); the rank-1 all-ones
    corrections fold into shipped U1'/V1'/U0' for free.  Halves the
    fp8 quantization error of the A operand.
  * U1/V1 shipped fp8 (x16) instead of bf16: 6 MB/t instead of 7.
    Their quantization error passes through A (row-sums ~0.5 on a
    zero-mean vector -> ~30x shrink), so the output impact is tiny.
  * Prologue in strict NEED-ORDER on the single sync-queue HWDGE
    (14 right-sized pieces; each dma_start costs ~0.6-1.2 us of queue
    time on ring credits, so piece count matters as much as bytes).
    No gpsimd/SWDGE transfers in the prologue window - they have no
    ordering vs HWDGE and steal ~40% of DMA bandwidth (v6's bug;
    first matmul fired at 17 us instead of ~10 us).
  * T_f starts with a q0-sweep across all 8 PSUM banks (8 matmuls per
    arriving j-pair piece) so the PE cannot outrun the t=0 DMA ramp;
    the q1..3 per-i passes space DVE drains ~675 ns apart.
  * 8 dummy matmuls on a memset scratch tile right after the engine
    preamble pre-warm the HAM clock gate (PE is held at 1.2 GHz until
    ~3.4 us of sustained activity) during the dead DMA-init window.
  * PSUM pool uses all 8 banks.

All spmm matmuls in fp8e4 with perf_mode=DoubleRow (contracts 2
K-planes per instruction via 3D APs [128, 2, free]; 216 ns warm
back-to-back spacing per 256-deep 512-wide matmul = the practical fp8
peak, 384 matmuls/core = 82.9 us PE floor).  The PE runs back-to-back
so the HAM clock gate stays at 2.4 GHz.

Drains are scalar_tensor_tensor on DVE: S8 = psum*2^-17 + U1'x16 (fp8),
osb = psum*2^-21 + U0' (bf16).  Scales (exact powers of two):
Ac8 = Ac*2^17 (|Ac|<2^-11 so |Ac8|<64), U1/U2/V1/V2 shipped x16,
U0' shipped unscaled bf16.

Sharding: t across 8 cores (3 each), zero collectives.
"""

import os
import sys

sys.path.insert(0, "/opt/trn_rl_repo")

import ml_dtypes
import numpy as np

import concourse.tile as tile
from concourse import bacc, mybir
from concourse.bass_utils import run_bass_kernel_spmd

B, T, N, D = 8, 24, 1024, 64
NCORES = 8
TPC = T // NCORES  # t-steps per core
NB = N // 128  # 128-row blocks of n
F32 = mybir.dt.float32
BF16 = mybir.dt.bfloat16
FP8 = mybir.dt.float8e4
BD = B * D
DR = mybir.MatmulPerfMode.DoubleRow
MULT = mybir.AluOpType.mult
ADD = mybir.AluOpType.add

SC_A = float(2.0**17)  # Ac8 = Ac * SC_A
SC_U = 16.0  # U1/V1/U2/V2 shipped * SC_U
C_S = float(2.0**-17)  # S8 = psum * C_S + U1x16  (= 16*S)
C_O = float(2.0**-21)  # osb = psum * C_O + U0

_cached = {}


def _build():
    if "nc" in _cached:
        return _cached["nc"]

    nc = bacc.Bacc("TRN2", target_bir_lowering=False, debug=False)
    # Host-pre-permuted layouts (see prep_in_maps).
    dAF = nc.dram_tensor("AFP", [TPC, 128, 2, NB, N], FP8, kind="ExternalInput")
    dUV8 = nc.dram_tensor("UV8P", [TPC, 128, 2, NB, BD], FP8, kind="ExternalInput")
    dUV1 = nc.dram_tensor("UV1P", [TPC, 128, 2, NB, BD], FP8, kind="ExternalInput")
    dU0 = nc.dram_tensor("U0P", [TPC, 128, NB, BD], BF16, kind="ExternalInput")
    dOUT = nc.dram_tensor("out", [TPC, 128, NB, BD], BF16, kind="ExternalOutput")

    with tile.TileContext(nc) as tc:
        with (
            tc.tile_pool(name="amat", bufs=2) as apool,
            tc.tile_pool(name="uv8", bufs=2) as uv8pool,
            tc.tile_pool(name="uv1", bufs=2) as uv1pool,
            tc.tile_pool(name="u0t", bufs=2) as u0pool,
            tc.tile_pool(name="sfb", bufs=2) as spool,
            tc.tile_pool(name="osb", bufs=2) as opool,
            tc.tile_pool(name="sps", bufs=8, space="PSUM") as sps,
        ):
            afs, uv8s, uv1s, u0s = {}, {}, {}, {}

            # ---------------- PE pre-warm --------------------------------
            # The HAM clock gate holds the PE at 1.2 GHz until it has been
            # busy ~3.4 us; real data lands ~4.5 us after the engine
            # preamble.  Burn the dead window on 8 dummy matmuls against a
            # memset scratch tile (ending right as the first real piece
            # arrives) so the real stream runs at 2.4 GHz from the start.
            warm = apool.tile([128, 2, BD], FP8, tag="warm", name="warmup")
            wps = sps.tile([128, BD], F32, tag="sps", name="warmps")
            nc.gpsimd.memset(warm[:], 0.0)
            for _ in range(8):
                nc.tensor.matmul(
                    wps[:],
                    warm[:, :, 0:128],
                    warm[:],
                    start=True,
                    stop=True,
                    perf_mode=DR,
                )

            def alloc_t(t):
                afs[t] = apool.tile([128, 2, NB, N], FP8, tag="af", name=f"af{t}")
                uv8s[t] = uv8pool.tile(
                    [128, 2, NB, BD], FP8, tag="uv8", name=f"uv8{t}"
                )
                uv1s[t] = uv1pool.tile(
                    [128, 2, NB, BD], FP8, tag="uv1", name=f"uv1{t}"
                )
                u0s[t] = u0pool.tile([128, NB, BD], BF16, tag="u0", name=f"u0{t}")

            # ---------------- prologue: t=0 strictly in need-order --------
            # ONE in-order HWDGE stream (sync queue) so no later tensor can
            # steal DMA bandwidth from an earlier-needed piece.  The first
            # T_f matmul group fires after just uv8-piece0 + a 32 KB af head
            # (the i=0 columns of j-pair 0); T_f reuses the same 1.5 MB for
            # all 8 i-groups, so DMA races ahead into the backward half
            # during T_f.
            alloc_t(0)
            # head: exactly what MM(dir0, i=0, q=0) reads -> earliest start
            # Each sync-queue dma_start occupies the queue ~0.6-0.75 us
            # (issue + HWDGE ring credits), so the piece COUNT is as
            # costly as the bytes: 14 pieces total, sized so each lands
            # just before its first consumer.
            nc.sync.dma_start(
                uv8s[0][:, 0, 0:2], dUV8.ap()[0, :, 0, 0:2]
            )
            nc.sync.dma_start(
                afs[0][:, 0, 0:2], dAF.ap()[0, :, 0, 0:2]
            )
            nc.sync.dma_start(uv8s[0][:, 0, 2:], dUV8.ap()[0, :, 0, 2:])
            for q in range(1, NB // 2):
                nc.sync.dma_start(
                    afs[0][:, 0, 2 * q : 2 * q + 2],
                    dAF.ap()[0, :, 0, 2 * q : 2 * q + 2],
                )
            # Only the i<4 half of U1' before the backward pieces: its
            # consumers are the T_f drains (DVE), which have slack; this
            # pulls afb01 — the piece T_b actually stalls on — ~2 chain
            # slots (~1 MB) earlier.
            nc.sync.dma_start(uv1s[0][:, 0, 0:4], dUV1.ap()[0, :, 0, 0:4])
            nc.sync.dma_start(uv8s[0][:, 1, 0:4], dUV8.ap()[0, :, 1, 0:4])
            nc.sync.dma_start(
                afs[0][:, 1, 0:4], dAF.ap()[0, :, 1, 0:4]
            )
            nc.sync.dma_start(uv1s[0][:, 0, 4:], dUV1.ap()[0, :, 0, 4:])
            nc.sync.dma_start(uv8s[0][:, 1, 4:], dUV8.ap()[0, :, 1, 4:])
            nc.sync.dma_start(
                afs[0][:, 1, 4:], dAF.ap()[0, :, 1, 4:]
            )
            nc.sync.dma_start(uv1s[0][:, 1], dUV1.ap()[0, :, 1])  # V1'x16
            nc.sync.dma_start(u0s[0][:], dU0.ap()[0])

            for t in range(TPC):
                af, uv8, uv1, u0 = afs[t], uv8s[t], uv1s[t], u0s[t]
                osb = opool.tile([128, NB, BD], BF16, tag="osb")
                sfb = spool.tile([128, 2, NB, BD], FP8, tag="sfb")
                have_next = t + 1 < TPC
                if have_next:
                    alloc_t(t + 1)

                # ---- T_f: S8[0] = 16*(U1' + A @ U2) -------------------
                # q0 is a sweep over all 8 PSUM banks: 8 matmuls per
                # arriving j-pair piece, so even a warm (2.4 GHz) PE can't
                # outrun the t=0 DMA ramp; the q1..3 per-i passes then
                # space the drains ~675 ns apart (>= DVE drain time), so
                # the DVE never backlogs into T_b/FB.
                psf = [
                    sps.tile([128, BD], F32, tag="sps", name=f"psf{t}_{i}")
                    for i in range(NB)
                ]
                for i in range(NB):
                    nc.tensor.matmul(
                        psf[i][:],
                        af[:, 0, 0:2, i * 128 : (i + 1) * 128],
                        uv8[:, 0, 0:2, :],
                        start=True,
                        stop=False,
                        perf_mode=DR,
                    )
                for i in range(NB):
                    for q in range(1, NB // 2):
                        nc.tensor.matmul(
                            psf[i][:],
                            af[:, 0, 2 * q : 2 * q + 2, i * 128 : (i + 1) * 128],
                            uv8[:, 0, 2 * q : 2 * q + 2, :],
                            start=False,
                            stop=(q == NB // 2 - 1),
                            perf_mode=DR,
                        )
                    nc.vector.scalar_tensor_tensor(
                        sfb[:, 0, i], psf[i][:], C_S, uv1[:, 0, i], MULT, ADD
                    )

                # ---- T_b: S8[1] = 16*(V1' + A^T @ V2) -----------------
                for i in range(NB):
                    if i == 0 and have_next:
                        # t+1 prefetch rides the SAME in-order HWDGE
                        # queue: it cannot start before t's (and t=0
                        # prologue's) earlier-needed pieces finish.
                        # (gpsimd/SWDGE has no ordering vs HWDGE and
                        # was measured stealing ~40% of prologue BW.)
                        nc.sync.dma_start(afs[t + 1][:], dAF.ap()[t + 1])
                        nc.sync.dma_start(
                            uv8s[t + 1][:, 0], dUV8.ap()[t + 1, :, 0]
                        )
                        nc.sync.dma_start(
                            uv1s[t + 1][:, 0], dUV1.ap()[t + 1, :, 0]
                        )
                    ps = sps.tile([128, BD], F32, tag="sps")
                    for q in range(NB // 2):
                        nc.tensor.matmul(
                            ps[:],
                            af[:, 1, 2 * q : 2 * q + 2, i * 128 : (i + 1) * 128],
                            uv8[:, 1, 2 * q : 2 * q + 2, :],
                            start=(q == 0),
                            stop=(q == NB // 2 - 1),
                            perf_mode=DR,
                        )
                    nc.vector.scalar_tensor_tensor(
                        sfb[:, 1, i], ps[:], C_S, uv1[:, 1, i], MULT, ADD
                    )

                # ---- FB: osb = U0' + A @ S_f + A^T @ S_b ----
                for i in range(NB):
                    if i == 0 and have_next:
                        nc.sync.dma_start(uv8s[t + 1][:, 1], dUV8.ap()[t + 1, :, 1])
                        nc.sync.dma_start(uv1s[t + 1][:, 1], dUV1.ap()[t + 1, :, 1])
                        nc.sync.dma_start(u0s[t + 1][:], dU0.ap()[t + 1])
                    ps = sps.tile([128, BD], F32, tag="sps")
                    for j in range(NB):
                        nc.tensor.matmul(
                            ps[:],
                            af[:, :, j, i * 128 : (i + 1) * 128],
                            sfb[:, :, j, :],
                            start=(j == 0),
                            stop=(j == NB - 1),
                            perf_mode=DR,
                        )
                    nc.vector.scalar_tensor_tensor(
                        osb[:, i], ps[:], C_O, u0[:, i], MULT, ADD
                    )
                    # store incrementally so the kernel tail only waits on
                    # the last 1-2 blocks
                    if i == 3:
                        nc.sync.dma_start(dOUT.ap()[t, :, 0:4], osb[:, 0:4])
                    elif i == 5:
                        nc.sync.dma_start(dOUT.ap()[t, :, 4:6], osb[:, 4:6])
                    elif i == 6:
                        nc.sync.dma_start(dOUT.ap()[t, :, 6:7], osb[:, 6:7])
                    elif i == 7:
                        nc.sync.dma_start(dOUT.ap()[t, :, 7:8], osb[:, 7:8])

    nc.compile()
    _cached["nc"] = nc
    return nc


def _uvperm(X):
    """[b, t(core-slice), n, d] -> [t, 128, NB, B*D] with
    out[t, p, i, b*64+d] = X[b, t, i*128+p, d]."""
    tpc = X.shape[1]
    return np.ascontiguousarray(
        X.transpose(1, 2, 0, 3)
        .reshape(tpc, NB, 128, B, D)
        .transpose(0, 2, 1, 3, 4)
        .reshape(tpc, 128, NB, BD)
    )


def _prep_core(UVall, A8, AT8, U0, c):
    ts = slice(c * TPC, (c + 1) * TPC)
    # AFP[t, p, dir, j, c] = (dir==0 ? Ac^T : Ac)[j*128+p, c] * 2^17 (fp8)
    AF = np.stack(
        [
            AT8[ts].reshape(TPC, NB, 128, N),
            A8[ts].reshape(TPC, NB, 128, N),
        ],
        axis=2,
    )  # [t, j, dir, p, col]
    AF = np.ascontiguousarray(AF.transpose(0, 3, 2, 1, 4))  # [t, p, dir, j, col]
    U1, U2, V1, V2 = (UVall[k][:, ts] for k in range(4))
    f8 = mybir.dt.np(FP8)
    # stack at axis=2: [t, 128, 2(slot), NB, BD]
    UV8 = np.ascontiguousarray(np.stack([_uvperm(U2), _uvperm(V2)], axis=2))
    UV1 = np.ascontiguousarray(np.stack([_uvperm(U1), _uvperm(V1)], axis=2))
    U0P = _uvperm(U0[:, ts])
    bf = ml_dtypes.bfloat16
    return {
        "AFP": AF,
        "UV8P": UV8.astype(f8),
        "UV1P": UV1.astype(f8),
        "U0P": U0P.astype(bf),
    }


def prep_in_maps(H, A, Wf, Wb, bias):
    H = np.ascontiguousarray(np.asarray(H, dtype=np.float32))
    A = np.ascontiguousarray(np.asarray(A, dtype=np.float32))
    Wf = np.asarray(Wf, dtype=np.float32)
    Wb = np.asarray(Wb, dtype=np.float32)
    bias = np.asarray(bias, dtype=np.float32)

    # ---- mean-center A; the all-ones rank-1 part folds into U1/V1/U0 ----
    mA = A.mean(axis=(1, 2), keepdims=True)  # [T,1,1]
    Ac = A - mA  # zero-mean, |Ac| < 1/N
    mAt = mA[:, 0, 0]  # [T]

    f8 = mybir.dt.np(FP8)
    A8 = (Ac * SC_A).astype(f8)
    AT8 = np.ascontiguousarray((Ac * SC_A).transpose(0, 2, 1)).astype(f8)

    U0 = (H @ (Wf[0] + Wb[0]) + bias).astype(np.float32)  # unscaled
    U1 = H @ Wf[1]
    U2 = H @ Wf[2]
    V1 = H @ Wb[1]
    V2 = H @ Wb[2]

    # Rank-1 corrections (J = all-ones):  A = mA*J + Ac
    #   S_f = U1 + A U2  = (U1 + mA*colsum(U2)) + Ac U2     -> ship U1'
    #   S_b = V1 + A^T V2 = (V1 + mA*colsum(V2)) + Ac^T V2  -> ship V1'
    #   out += mA*(colsum(S_f) + colsum(S_b))               -> fold into U0'
    # colsum(S_f) = colsum(U1') + (1^T Ac) @ U2  (host, exact f32)
    csU2 = U2.sum(axis=2)  # [B,T,D]
    csV2 = V2.sum(axis=2)
    mAb = mAt[None, :, None]  # [1,T,1]
    U1p = U1 + (mAb * csU2)[:, :, None, :]
    V1p = V1 + (mAb * csV2)[:, :, None, :]
    colAc = Ac.sum(axis=1)  # [T,N]: (1^T Ac)_k = sum_j Ac[j,k]
    rowAc = Ac.sum(axis=2)  # [T,N]: (Ac 1)_i  = sum_j Ac[i,j]
    csSf = U1p.sum(axis=2) + np.einsum("tk,btkd->btd", colAc, U2)
    csSb = V1p.sum(axis=2) + np.einsum("ti,btid->btd", rowAc, V2)
    U0p = U0 + (mAb * (csSf + csSb))[:, :, None, :]

    UVall = [
        (X * SC_U).astype(np.float32) for X in (U1p, U2, V1p, V2)
    ]  # U1', U2, V1', V2 (x16)

    return [_prep_core(UVall, A8, AT8, U0p, c) for c in range(NCORES)]


def _postprocess(res):
    # osb = psum*2^-21 + U0' is already the final unscaled output.
    outp = np.concatenate(
        [np.asarray(res.results[c]["out"]) for c in range(NCORES)], axis=0
    ).astype(np.float32)
    out = (
        outp.reshape(T, 128, NB, B, D)
        .transpose(3, 0, 2, 1, 4)  # [b, t, i, p, d]
        .reshape(B, T, N, D)
    )
    return np.ascontiguousarray(out)


def kernel(H, A, Wf, Wb, bias):
    nc = _build()
    in_maps = prep_in_maps(H, A, Wf, Wb, bias)
    res = run_bass_kernel_spmd(nc, in_maps, core_ids=list(range(NCORES)))
    return _postprocess(res)
